# revision 34
# baseline (speedup 1.0000x reference)
"""Causal self-attention Trainium2 kernel (8-core SPMD), v2.

Sharding: 8 cores = 4 batches x 2 head-groups (tensor parallel over heads).
Each core computes, for its batch b and its 8 heads:
  QKV projection (transposed layouts), causal flash-style attention without
  max-subtraction (scores are O(+-10), safe in fp32), and a partial output
  projection over its head-group's rows of W_proj.  The host sums the two
  partial outputs per batch (the "all-reduce" of the hint, done host-side).

v2 restructure vs v1:
  - Cross-phase interleaving: QKV(qt+1) and proj(qt-1) matmuls are emitted
    as filler units inside attention(qt)'s kc loop, so the PE fills the
    gaps left by the ACT-paced exp chain.
  - Diagonal trimming: score matmuls and exp only cover the causally valid
    column range [s:512]; affine_select (full range) zero-fills the rest.
  - Merged normalization: both heads of a pair share one [128,512] multiply.
  - fc rotation in proj accumulation so tail proj units depend on different
    head-pairs (shorter dependency tail).
  - Chunked x^T DMA + early weight DMAs + ACT exp-table preload.

Device layouts (per core):
  x       [T, C]    this batch's activations (host-pre-transposed to xt)
  xT      [C, T]
  Q^T,K^T [f, t]    f = head-major features (head pair per 128-chunk)
  V_ext   [t, 8*65] per head: 64 V columns + a ones column (softmax denom
                    falls out of the attn@V matmul for free)
  S^T     [k, q]    scores transposed; softmax denom = ones-row of V_ext
  y^T     [f, t]    normalized attention output, feeds W_proj matmul
  out     [T, C]    partial projection output (host adds the two halves)
"""

import numpy as np

import concourse.bass as bass
import concourse.mybir as mybir
import concourse.tile as tile
from concourse import bacc
from concourse.bass_utils import run_bass_kernel_spmd

F32 = mybir.dt.float32
P = 128
NEG = -1.0e30


def build_nc(T=2048, C=1024, n_loc_heads=8, debug=False, reps=1,
             mm_dt=mybir.dt.float32r, narrow=True, trim=True):
    """Build the per-core SPMD program. T must be a multiple of 512."""
    D = 64
    HL = n_loc_heads              # local heads (8)
    FQK = HL * D                  # 512: Q (and K) features per core
    NQT = T // 512                # q-tiles of 512
    NTC = T // P                  # t-chunks of 128
    NCO = C // P                  # contraction chunks (8)
    NM = 2 * FQK // P             # Q+K feature chunks (8)
    NFC = FQK // P                # y^T feature chunks (4)
    NCT = C // 512                # output column tiles (2)
    Exp = mybir.ActivationFunctionType.Exp
    MDT = mm_dt
    BDT = mybir.dt.bfloat16       # score/attn@V operand dtype

    nc = bacc.Bacc(target_bir_lowering=False, debug=debug)
    xt = nc.dram_tensor("xt", [C, T], mm_dt, kind="ExternalInput")
    wqk = nc.dram_tensor("wqk", [2 * FQK // P, P, C // P, P], mm_dt,
                         kind="ExternalInput")
    wv = nc.dram_tensor("wv", [C, FQK], mm_dt, kind="ExternalInput")
    wpr = nc.dram_tensor("wpr", [FQK, C], mm_dt, kind="ExternalInput")
    bqk = nc.dram_tensor("bqk", [P, NM], F32, kind="ExternalInput")
    bv = nc.dram_tensor("bv", [P, FQK], F32, kind="ExternalInput")
    out = nc.dram_tensor("out", [T, C], F32, kind="ExternalOutput")

    with tile.TileContext(nc) as tc:
        with (
            tc.tile_pool(name="const", bufs=1) as cpool,
            tc.tile_pool(name="persist", bufs=1) as ppool,
            tc.tile_pool(name="xt", bufs=2) as xtp,
            tc.tile_pool(name="qt", bufs=2) as qtp,
            tc.tile_pool(name="yt", bufs=2) as ytp,
            tc.tile_pool(name="pt", bufs=3) as ptp,
            tc.tile_pool(name="yx", bufs=8) as yxp,
            tc.tile_pool(name="wqk", bufs=4) as wqkp,
            tc.tile_pool(name="oout", bufs=2) as outp,
            tc.tile_pool(name="dnm", bufs=2) as dnp,
            tc.tile_pool(name="mm", bufs=2, space="PSUM") as mmp,
            tc.tile_pool(name="sp", bufs=2, space="PSUM") as spp,
            tc.tile_pool(name="yps", bufs=2, space="PSUM") as ypp,
        ):
            # ---- constants ----
            ones_sb = cpool.tile([P, HL, 1], F32, tag="ones")
            nc.vector.memset(ones_sb[:], 1.0)
            # preload the exp table set so the first real exp doesn't stall
            scratch = cpool.tile([1, 8], F32, tag="scratch")
            nc.scalar.activation(scratch[:], ones_sb[0:1, 0:8, 0], Exp)
            bqk_sb = cpool.tile([P, NM], F32, tag="bqk")
            nc.sync.dma_start(bqk_sb[:], bqk[:, :])
            # 8 denominator rows parked at legal partition bases
            # (0/32/64/96) x two free-dim halves; one batched reciprocal
            # per q-tile covers them all (DVE cost is per-partition free
            # length). memset once so the recip never reads junk.
            dens8 = cpool.tile([P, 2, 512], F32, tag="dens")
            nc.vector.memset(dens8[:], 1.0)
            bv_sb = cpool.tile([P, FQK], F32, tag="bv")

            # ---- persistent tensors ----
            KT = ppool.tile([P, NFC, T], BDT, tag="KT")
            VE = ppool.tile([P, NTC, HL * (D + 1)], BDT, tag="VE")
            wv_sb = ppool.tile([P, NCO, FQK], MDT, tag="wv")
            wpr_sb = ppool.tile([P, NFC, C], MDT, tag="wpr")

            # V/proj weight prefetch on the ACT engine's DMA queues --
            # parallel with the sync engine's xt/wqk streams, early
            # enough to land before the V matmuls / first proj units.
            nc.scalar.dma_start(bv_sb[:], bv[:, :])
            nc.scalar.dma_start(
                wv_sb[:], wv.rearrange("(co ci) n -> ci co n", ci=P))
            nc.scalar.dma_start(
                wpr_sb[:], wpr.rearrange("(fo fi) n -> fi fo n", fi=P))

            def emit_xt_dma(qt, halves=(0, 1)):
                q0 = qt * 512
                xTt = xtp.tile([P, NCO, 512], MDT, tag="xT")
                for h in halves:
                    nc.sync.dma_start(
                        xTt[:, 4 * h:4 * h + 4, :],
                        xt[512 * h:512 * (h + 1), q0:q0 + 512].rearrange(
                            "(co ci) t -> ci co t", ci=P))
                return xTt

            def qkv_units(qt, xTt, first=False, after_wqk0=None):
                """12 filler units: 8 QK m-chunks + 4 V t-chunks."""
                q0 = qt * 512
                QTt = qtp.tile([P, NFC, 512], BDT, tag="QTt")
                units = []
                for m in range(NM):
                    def u(m=m, QTt=QTt, xTt=xTt, q0=q0):
                        ps = mmp.tile([P, 512], F32, tag="mm")
                        wt = wqkp.tile([P, NCO, P], MDT, tag="wqk")
                        nc.sync.dma_start(wt[:], wqk[m])
                        if m == 0 and after_wqk0 is not None:
                            after_wqk0()
                        for co in range(NCO):
                            nc.tensor.matmul(ps[:], wt[:, co, :],
                                             xTt[:, co, :],
                                             start=(co == 0),
                                             stop=(co == NCO - 1))
                        if m < NFC:
                            dst = QTt[:, m, :]
                        else:
                            dst = KT[:, m - NFC, q0:q0 + 512]
                        nc.vector.tensor_scalar_add(dst, ps[:],
                                                    bqk_sb[:, m:m + 1])
                    units.append(u)
                for tc_i in range(4):
                    def u(tc_i=tc_i, xTt=xTt, qt=qt):
                        ps = mmp.tile([P, 512], F32, tag="mm")
                        for co in range(NCO):
                            nc.tensor.matmul(
                                ps[:], xTt[:, co, tc_i * P:(tc_i + 1) * P],
                                wv_sb[:, co, :],
                                start=(co == 0), stop=(co == NCO - 1))
                        tci = qt * 4 + tc_i
                        vev = VE[:, tci, :].rearrange("p (h e) -> p h e",
                                                      e=D + 1)
                        nc.vector.tensor_add(
                            vev[:, :, :D],
                            ps[:].rearrange("p (h d) -> p h d", d=D),
                            bv_sb[:].rearrange("p (h d) -> p h d", d=D))
                        nc.vector.tensor_copy(vev[:, :, D:D + 1], ones_sb[:])
                    units.append(u)
                return QTt, units

            def proj_units(qt, yTt):
                """8 filler units; fc accumulation order rotated per unit so
                consecutive units finish on different head-pairs."""
                q0 = qt * 512
                units = []
                idx = 0
                for tc_i in range(4):
                    for ct in range(NCT):
                        def u(tc_i=tc_i, ct=ct, r0=idx % NFC, yTt=yTt, q0=q0):
                            ps = mmp.tile([P, 512], F32, tag="mm")
                            for j in range(NFC):
                                fc = (r0 + j) % NFC
                                nc.tensor.matmul(
                                    ps[:],
                                    yTt[:, fc, tc_i * P:(tc_i + 1) * P],
                                    wpr_sb[:, fc, ct * 512:(ct + 1) * 512],
                                    start=(j == 0), stop=(j == NFC - 1))
                            ot = outp.tile([P, 512], F32, tag="oout")
                            # ACT copy (PSUM->SBUF): keeps the proj
                            # completion chain off the DVE queue (which
                            # carries the big reciprocals); b_proj is
                            # added host-side with the partial sum.
                            nc.scalar.copy(ot[:], ps[:])
                            nc.sync.dma_start(
                                out[q0 + tc_i * P:q0 + (tc_i + 1) * P,
                                    ct * 512:(ct + 1) * 512], ot[:])
                        units.append(u)
                        idx += 1
                return units

            def attention(qt, QTt, early, inloop=(), late=()):
                """Attention over this q-tile, software-pipelined: the score
                matmuls run one iteration ahead of exp/attn@V so the PE can
                execute filler units while ACT computes the exp."""
                nk = 4 * (qt + 1)
                yTt = ytp.tile([P, NFC, 512], MDT, tag="yTt")
                iters = [(ch, kc) for ch in range(NFC) for kc in range(nk)]
                total = len(iters)
                sp_tiles = {}
                norm_jobs = []

                def emit_scores(i):
                    ch, kc = iters[i]
                    diag = kc >= 4 * qt
                    s = 128 * kc - 512 * qt if diag else 0
                    if not narrow:
                        s = 0
                    sp2 = spp.tile([P, 1024], F32, tag="sp")
                    nc.tensor.matmul(
                        sp2[:, s:512],
                        KT[0:64, ch, kc * P:(kc + 1) * P],
                        QTt[0:64, ch, s:512],
                        start=True, stop=True, tile_position=(0, 0))
                    nc.tensor.matmul(
                        sp2[:, 512 + s:1024],
                        KT[64:128, ch, kc * P:(kc + 1) * P],
                        QTt[64:128, ch, s:512],
                        start=True, stop=True, tile_position=(64, 0))
                    sp_tiles[i] = (sp2, s, diag)

                fe = fl = 0
                emit_scores(0)
                ypsA = ypsB = None
                for i, (ch, kc) in enumerate(iters):
                    if kc == 0:
                        ypsA = ypp.tile([P, 512], F32, tag="yps")
                        ypsB = ypp.tile([P, 512], F32, tag="yps")
                    if i + 1 < total:
                        emit_scores(i + 1)
                    sp2, s, diag = sp_tiles.pop(i)
                    if not narrow:
                        s = 0
                    pt_t = ptp.tile([P, 1024], BDT, tag="pt")
                    pt_v = pt_t[:].rearrange("p (h q) -> p h q", h=2)
                    sp_v = sp2[:].rearrange("p (h q) -> p h q", h=2)
                    if diag and s > 0:
                        nc.scalar.activation(pt_v[:, :, s:512],
                                             sp_v[:, :, s:512], Exp,
                                             scale=0.125)
                        # columns [0:s) are fully masked and never read:
                        # the attn@V matmuls below are trimmed to [s:512).
                        if not trim:
                            nc.vector.memset(pt_t[:, 0:s], 0.0)
                            nc.vector.memset(pt_t[:, 512:512 + s], 0.0)
                    else:
                        nc.scalar.activation(pt_t[:], sp2[:], Exp,
                                             scale=0.125)
                    if diag:
                        # zero the invalid (k > q) triangle: it spans only
                        # the 128 columns [s, s+128) (row p invalid before
                        # col s+p), so select on that band alone.
                        nc.gpsimd.affine_select(
                            out=pt_v[:, :, s:s + 128],
                            in_=pt_v[:, :, s:s + 128],
                            compare_op=mybir.AluOpType.is_ge,
                            fill=0.0, base=0, channel_multiplier=-1,
                            pattern=[[0, 2], [1, 128]])
                    # in-loop fillers: only DMA-free units (proj of an
                    # old q-tile) -- units with their own weight DMAs race
                    # on HW when executed right behind the transfer
                    ti = ((i + 1) * len(inloop)) // total
                    while fe < ti:
                        inloop[fe]()
                        fe += 1
                    half = total // 2
                    if i >= half:
                        tl = ((i + 1 - half) * len(late)) // (total - half)
                        while fl < tl:
                            late[fl]()
                            fl += 1
                    hA, hB = 2 * ch, 2 * ch + 1
                    # diag chunks: columns [0:s) were never written (fully
                    # masked) so the accumulation is trimmed to [s:512).
                    st = s if trim else 0
                    nc.tensor.matmul(
                        ypsA[:D + 1, st:512],
                        VE[:, kc, hA * (D + 1):(hA + 1) * (D + 1)],
                        pt_t[:, st:512],
                        start=(kc == 0), stop=(kc == nk - 1),
                        skip_group_check=True)
                    nc.tensor.matmul(
                        ypsB[:D + 1, st:512],
                        VE[:, kc, hB * (D + 1):(hB + 1) * (D + 1)],
                        pt_t[:, 512 + st:1024],
                        start=(kc == 0), stop=(kc == nk - 1),
                        skip_group_check=True)
                    if kc == nk - 1:
                        # stash unnormalized y + its denominator row; the
                        # reciprocal is batched for all 8 heads at the tail
                        # (one [8,512] DVE op costs the same as [1,512]).
                        for po, yps in ((0, ypsA), (64, ypsB)):
                            pb = 32 * (2 * (ch % 2) + po // 64)
                            half = ch // 2
                            yext = yxp.tile([D, 512], F32, tag="yext")
                            nc.vector.tensor_copy(yext[:], yps[:D, :])
                            nc.vector.tensor_copy(
                                dens8[pb:pb + 1, half, :],
                                yps[D:D + 1, :])
                            norm_jobs.append((ch, po, pb, half, yext))
                        if ch % 2 == 1:
                            # batched reciprocal for this ch-pair's half
                            # (4 dens at bases 0/32/64/96); norms for two
                            # chs then overlap the remaining attention.
                            # partition_broadcast reads physical
                            # partition 0 only, so each recip'd row is
                            # staged down to a base-0 tile first.
                            nc.vector.reciprocal(
                                dens8[:, ch // 2, :],
                                dens8[:, ch // 2, :])
                            for c2, po, pb, hf, yext in norm_jobs:
                                rd = dnp.tile([1, 512], F32, tag="rd")
                                nc.vector.tensor_copy(
                                    rd[:], dens8[pb:pb + 1, hf, :])
                                repb = dnp.tile([64, 512], F32,
                                                tag="rep")
                                nc.gpsimd.partition_broadcast(
                                    repb[:], rd[:])
                                nc.vector.tensor_mul(
                                    yTt[po:po + 64, c2, :], yext[:],
                                    repb[:])
                            norm_jobs = []
                while fe < len(inloop):
                    inloop[fe]()
                    fe += 1
                while fl < len(late):
                    late[fl]()
                    fl += 1
                for u in early:
                    u()
                return yTt

            def interleave(a, b):
                res = []
                n = max(len(a), len(b))
                for i in range(n):
                    if i < len(a):
                        res.append(a[i])
                    if i < len(b):
                        res.append(b[i])
                return res

            # ---- prologue: QKV for (rep 0, qt 0) emitted directly ----
            # xt chunk A first, then wqk[0] (inside unit 0), then chunk B:
            # the first matmuls need only chunk A + wqk[0].
            xTt = emit_xt_dma(0, halves=(0,))

            def _rest_of_xt(xTt=xTt):
                nc.sync.dma_start(
                    xTt[:, 4:8, :],
                    xt[512:1024, 0:512].rearrange("(co ci) t -> ci co t",
                                                  ci=P))

            QTt, units = qkv_units(0, xTt, first=True, after_wqk0=_rest_of_xt)
            for u in units:
                u()

            pending = None                       # (yTt, qt) awaiting proj
            for rep in range(reps):
                for qt in range(NQT):
                    nqt, nrep = qt + 1, rep
                    if nqt == NQT:
                        nqt, nrep = 0, rep + 1
                    qunits = []
                    QTt2 = None
                    if nrep < reps:
                        xTt2 = emit_xt_dma(nqt)
                        QTt2, qunits = qkv_units(nqt, xTt2)
                    punits = []
                    if pending is not None:
                        punits = proj_units(pending[1], pending[0])
                    munits, vunits = qunits[:NM], qunits[NM:]
                    if qt >= 2:
                        yTt = attention(qt, QTt, munits, inloop=punits,
                                        late=vunits)
                    elif qt == 1:
                        yTt = attention(qt, QTt, interleave(munits, punits),
                                        late=vunits)
                    else:
                        yTt = attention(qt, QTt, interleave(qunits, punits))
                    pending = (yTt, qt)
                    QTt = QTt2

            for u in proj_units(pending[1], pending[0]):
                u()

    nc.compile()
    return nc


_CACHE = {}


def _get_nc():
    if "nc" not in _CACHE:
        _CACHE["nc"] = build_nc()
    return _CACHE["nc"]


def make_in_maps(x, W_attn, b_attn, W_proj, b_proj, B=4, C=1024):
    x = np.ascontiguousarray(np.asarray(x, dtype=np.float32))
    W_attn = np.asarray(W_attn, dtype=np.float32)
    b_attn = np.asarray(b_attn, dtype=np.float32)
    W_proj = np.asarray(W_proj, dtype=np.float32)
    b_proj = np.asarray(b_proj, dtype=np.float32)
    in_maps = []
    for core in range(2 * B):
        b, hg = core // 2, core % 2
        s = slice(hg * 512, (hg + 1) * 512)
        wqk_flat = np.concatenate(
            [W_attn[:, s], W_attn[:, C + hg * 512:C + (hg + 1) * 512]],
            axis=1)  # [C, 1024]
        # [m, ci, co, f]: per m-chunk one contiguous [128, 8, 128] block
        # (4KB per partition row -> large DMA descriptors).
        # wqk_flat[co*128+ci, m*128+f] -> reshape [co, ci, m, f]
        wqk_c = np.ascontiguousarray(
            wqk_flat.reshape(8, 128, 8, 128).transpose(2, 1, 0, 3))
        wv_c = np.ascontiguousarray(W_attn[:, 2 * C + hg * 512:
                                           2 * C + (hg + 1) * 512])
        wpr_c = np.ascontiguousarray(W_proj[hg * 512:(hg + 1) * 512, :])
        # b_proj is added host-side together with the partial-output sum
        bqk_vec = np.concatenate([b_attn[s], b_attn[C + hg * 512:
                                                    C + (hg + 1) * 512]])
        bqk_c = np.ascontiguousarray(bqk_vec.reshape(8, 128).T)
        bv_c = np.ascontiguousarray(
            np.tile(b_attn[2 * C + hg * 512:2 * C + (hg + 1) * 512][None, :],
                    (128, 1)))
        in_maps.append({
            "xt": np.ascontiguousarray(x[b].T),
            "wqk": wqk_c, "wv": wv_c, "wpr": wpr_c,
            "bqk": bqk_c, "bv": bv_c,
        })
    return in_maps


def kernel(x, W_attn, b_attn, W_proj, b_proj):
    B, T, C = 4, 2048, 1024
    nc = _get_nc()
    in_maps = make_in_maps(x, W_attn, b_attn, W_proj, b_proj, B=B, C=C)
    res = run_bass_kernel_spmd(nc, in_maps, list(range(2 * B)))
    bp = np.asarray(b_proj, dtype=np.float32)
    out = np.empty((B, T, C), dtype=np.float32)
    for b in range(B):
        np.add(res.results[2 * b]["out"], res.results[2 * b + 1]["out"],
               out=out[b])
        out[b] += bp
    return out

